# revision 1
# baseline (speedup 1.0000x reference)
"""NeRF MLP forward pass on 8 Trainium2 NeuronCores (Bass/Tile).

Strategy: pure data parallel over rays. Each core processes 512 rays x 64
samples = 32768 points through the full MLP. Activations live transposed in
SBUF as [hidden, n_points] so every linear layer is a chain of
128x128 (stationary weight) x [128, 512] (moving activations) matmuls in
float32r (full-rate fp32 with 11-bit mantissa). Harmonic embeddings are
computed on-chip with Cody-Waite range reduction + the ScalarE Sin LUT.
"""

import sys

if '/opt/trn_rl_repo' not in sys.path:
    sys.path.insert(0, '/opt/trn_rl_repo')

import numpy as np

import concourse.bacc as bacc
import concourse.mybir as mybir
import concourse.tile as tile
from concourse.bass_utils import run_bass_kernel_spmd

F32 = mybir.dt.float32
F32R = mybir.dt.float32r
AF = mybir.ActivationFunctionType
ALU = mybir.AluOpType

N_CORES = 8
N_RAYS, S = 4096, 64
R_CORE = N_RAYS // N_CORES            # 512 rays per core
NPTS = R_CORE * S                     # 32768 points per core
# Points are ordered SAMPLE-major per core: point index = s * R_CORE + r.
# A 512-point sub-tile is then exactly one sample index across all rays,
# and the per-ray direction embedding broadcast is a contiguous block
# repeat.
F = 512                               # points per matmul (one PSUM bank)
FSUP = 2048                           # points per super-tile
NSUB = FSUP // F                      # 4
NSUP = NPTS // FSUP                   # 16
S_SUP = FSUP // R_CORE                # 4 samples per super-tile

H = 256
EMB_X = 63
EMB_D = 27

PI = float(np.pi)
INV2PI = float(1.0 / (2.0 * np.pi))
MAGIC = float(1.5 * 2 ** 23)
# Cody-Waite split of 2*pi: c1 exact in 9 mantissa bits, c2 in ~12, c3 rest.
_t = 2.0 * np.pi - 6.28125
_c2u = np.float32(_t).view(np.uint32) & np.uint32(0xFFFFF000)
CW1 = 6.28125
CW2 = float(_c2u.view(np.float32))
CW3 = float(np.float32(_t - float(_c2u.view(np.float32))))

# (name, kparts, out_chunks) ; kparts entries: (src, chunk_idx, ksize)
_XYZ_LAYERS = []
for li in range(8):
    if li == 0:
        kparts = [("E", 0, EMB_X)]
    elif li == 4:
        kparts = [("x", 0, 128), ("x", 1, 128), ("E", 0, EMB_X)]
    else:
        kparts = [("x", 0, 128), ("x", 1, 128)]
    _XYZ_LAYERS.append(kparts)

_cache = {}


def _build(nsup_exec=NSUP):
    """Build the bass program. nsup_exec > NSUP repeats super-tiles
    (st = i % NSUP) — used only for slope-based timing benchmarks."""
    key = ("nc", nsup_exec)
    if key in _cache:
        return _cache[key]

    nc = bacc.Bacc("TRN2", target_bir_lowering=False, debug=False,
                   num_devices=N_CORES)

    pts = nc.dram_tensor("pts", [3, NPTS], F32, kind="ExternalInput")
    dirs = nc.dram_tensor("dirs", [3, R_CORE], F32, kind="ExternalInput")
    w0 = nc.dram_tensor("w0", [EMB_X, 256], F32, kind="ExternalInput")
    wmid = {i: nc.dram_tensor(f"wmid{i}", [128, 512], F32, kind="ExternalInput")
            for i in range(1, 8)}
    w4e = nc.dram_tensor("w4e", [EMB_X, 256], F32, kind="ExternalInput")
    wfeat = nc.dram_tensor("wfeat", [128, 512], F32, kind="ExternalInput")
    wden = nc.dram_tensor("wden", [128, 2], F32, kind="ExternalInput")
    wd0 = nc.dram_tensor("wd0", [128, 256], F32, kind="ExternalInput")
    wd0e = nc.dram_tensor("wd0e", [EMB_D, 128], F32, kind="ExternalInput")
    wrgb = nc.dram_tensor("wrgb", [128, 3], F32, kind="ExternalInput")
    biases = nc.dram_tensor("biases", [128, 21], F32, kind="ExternalInput")
    consts = nc.dram_tensor("consts", [128, 4], F32, kind="ExternalInput")
    out = nc.dram_tensor("out", [4, NPTS], F32, kind="ExternalOutput")

    with tile.TileContext(nc) as tc:
        with (
            tc.tile_pool(name="wpool", bufs=1) as wpool,
            tc.tile_pool(name="epool", bufs=3) as epool,
            tc.tile_pool(name="spool", bufs=2) as spool,
            tc.tile_pool(name="apool", bufs=1) as apool,
            tc.tile_pool(name="opool", bufs=2) as opool,
            tc.tile_pool(name="psum", bufs=8, space="PSUM") as psum,
        ):
            # ---- load weights / constants (once) ----
            w0_t = wpool.tile([EMB_X, 256], F32R)
            nc.sync.dma_start(w0_t[:], w0[:].bitcast(F32R))
            wmid_t = {}
            for i in range(1, 8):
                wt = wpool.tile([128, 512], F32R, name=f"wmid{i}_t")
                nc.sync.dma_start(wt[:], wmid[i][:].bitcast(F32R))
                wmid_t[i] = wt
            w4e_t = wpool.tile([EMB_X, 256], F32R)
            nc.sync.dma_start(w4e_t[:], w4e[:].bitcast(F32R))
            wfeat_t = wpool.tile([128, 512], F32R)
            nc.sync.dma_start(wfeat_t[:], wfeat[:].bitcast(F32R))
            wden_t = wpool.tile([128, 2], F32R)
            nc.sync.dma_start(wden_t[:], wden[:].bitcast(F32R))
            wd0_t = wpool.tile([128, 256], F32R)
            nc.sync.dma_start(wd0_t[:], wd0[:].bitcast(F32R))
            wd0e_t = wpool.tile([EMB_D, 128], F32R)
            nc.sync.dma_start(wd0e_t[:], wd0e[:].bitcast(F32R))
            wrgb_t = wpool.tile([128, 3], F32R)
            nc.sync.dma_start(wrgb_t[:], wrgb[:].bitcast(F32R))
            b_t = wpool.tile([128, 21], F32)
            nc.sync.dma_start(b_t[:], biases[:])
            c_t = wpool.tile([128, 4], F32)
            nc.sync.dma_start(c_t[:], consts[:])
            zeros_t = wpool.tile([128, 1], F32)
            nc.any.memset(zeros_t[:], 0.0)

            def sincos(dst, scratch_p, scratch_k, freqshift, nrows):
                """dst[0:2*nrows] = [sin(a), cos(a)] with a = raw args
                replicated in both halves of scratch_p. freqshift is a
                [2*nrows, 2] fp32 AP: col0 per-row freq scale, col1 per-row
                shift (pi/2 for the cos half). Scratch is destroyed."""
                nc.vector.tensor_scalar(scratch_p[:], scratch_p[:],
                                        freqshift[:, 0:1], freqshift[:, 1:2],
                                        op0=ALU.mult, op1=ALU.add)
                nc.vector.tensor_scalar(scratch_k[:], scratch_p[:], INV2PI,
                                        MAGIC, op0=ALU.mult, op1=ALU.add)
                nc.vector.tensor_scalar(scratch_k[:], scratch_k[:], MAGIC,
                                        None, op0=ALU.subtract)
                nc.vector.cody_waite_cascade(scratch_p[:], scratch_p[:],
                                             scratch_k[:], CW1, CW2, CW3)
                nc.scalar.activation(dst[0:2 * nrows, :], scratch_p[:],
                                     AF.Sin, bias=zeros_t[0:2 * nrows, 0:1])

            # direction embedding tile (computed after emb_first below so the
            # xyz chain of super-tile 0 heads the in-order DVE/ACT queues)
            embd_rays = wpool.tile([EMB_D, R_CORE], F32R)
            pd = wpool.tile([24, R_CORE], F32)
            kd = wpool.tile([24, R_CORE], F32)

            # ---- per super-tile pipeline ----
            ev_counter = [0]

            def evict(psum_ap, out_ap, bias_ap):
                """relu(psum + bias) -> fp32r SBUF, alternating ACT/DVE."""
                if ev_counter[0] % 2 == 0:
                    nc.scalar.activation(out_ap, psum_ap, AF.Relu,
                                         bias=bias_ap)
                else:
                    nc.vector.tensor_scalar(out_ap, psum_ap, bias_ap, 0.0,
                                            op0=ALU.add, op1=ALU.max)
                ev_counter[0] += 1

            def emb_stages(st):
                """Generator emitting the harmonic-embedding pipeline for
                super-tile st one stage per next() call, so the serial DVE
                chain spreads through the in-order engine queues instead of
                landing as one burst. Final yield returns (E, embd)."""
                sl = slice(st * FSUP, (st + 1) * FSUP)
                P = spool.tile([60, FSUP], F32, name="P")
                K = spool.tile([60, FSUP], F32, name="K")
                for half in range(2):
                    for c in range(3):
                        nc.sync.dma_start(
                            P[half * 30 + c * 10:half * 30 + (c + 1) * 10, :],
                            pts[c:c + 1, sl].partition_broadcast(10))
                nc.vector.tensor_scalar(P[:], P[:], c_t[0:60, 0:1],
                                        c_t[0:60, 1:2],
                                        op0=ALU.mult, op1=ALU.add)
                yield None
                nc.vector.tensor_scalar(K[:], P[:], INV2PI, MAGIC,
                                        op0=ALU.mult, op1=ALU.add)
                yield None
                nc.vector.tensor_scalar(K[:], K[:], MAGIC, None,
                                        op0=ALU.subtract)
                yield None
                nc.vector.cody_waite_cascade(P[:], P[:], K[:], CW1, CW2, CW3)
                yield None
                E = epool.tile([EMB_X, FSUP], F32R, name="E")
                nc.scalar.activation(E[0:60, :], P[:], AF.Sin,
                                     bias=zeros_t[0:60, 0:1])
                nc.sync.dma_start(E[60:63, :], pts[:, sl].bitcast(F32R))
                # broadcast direction embedding to per-point
                embd = epool.tile([EMB_D, FSUP], F32R, name="embd")
                nc.sync.dma_start(
                    embd[:].rearrange("p (s r) -> p s r", s=S_SUP),
                    embd_rays[:].unsqueeze(1)
                    .broadcast_to([EMB_D, S_SUP, R_CORE]))
                yield (E, embd)

            def emb_first():
                """Super-tile 0 prologue: run the embedding pipeline in two
                column chunks so the PE can start layer 0 on the first
                512-point sub-tile ~5us earlier (element-wise identical)."""
                sl0 = slice(0, FSUP)
                P = spool.tile([60, FSUP], F32, name="P")
                K = spool.tile([60, FSUP], F32, name="K")
                for half in range(2):
                    for c in range(3):
                        nc.sync.dma_start(
                            P[half * 30 + c * 10:half * 30 + (c + 1) * 10, :],
                            pts[c:c + 1, sl0].partition_broadcast(10))
                E = epool.tile([EMB_X, FSUP], F32R, name="E")
                for a, b in ((0, F), (F, FSUP)):
                    Pv, Kv = P[:, a:b], K[:, a:b]
                    nc.vector.tensor_scalar(Pv, Pv, c_t[0:60, 0:1],
                                            c_t[0:60, 1:2],
                                            op0=ALU.mult, op1=ALU.add)
                    nc.vector.tensor_scalar(Kv, Pv, INV2PI, MAGIC,
                                            op0=ALU.mult, op1=ALU.add)
                    nc.vector.tensor_scalar(Kv, Kv, MAGIC, None,
                                            op0=ALU.subtract)
                    nc.vector.cody_waite_cascade(Pv, Pv, Kv, CW1, CW2, CW3)
                    nc.scalar.activation(E[0:60, a:b], Pv, AF.Sin,
                                         bias=zeros_t[0:60, 0:1])
                nc.sync.dma_start(E[60:63, :], pts[:, sl0].bitcast(F32R))
                return E

            E0 = emb_first()
            # ---- direction embedding per ray (once per core) ----
            for half in range(2):
                for c in range(3):
                    nc.sync.dma_start(
                        pd[half * 12 + c * 4:half * 12 + (c + 1) * 4, :],
                        dirs[c:c + 1, :].partition_broadcast(4))
            sincos(embd_rays, pd, kd, c_t[0:24, 2:4], 12)
            nc.sync.dma_start(embd_rays[24:27, :], dirs[:].bitcast(F32R))
            # st0 direction-embedding broadcast (must be emitted AFTER the
            # embd_rays writers: Tile tracks deps in emission order)
            embd0 = epool.tile([EMB_D, FSUP], F32R, name="embd")
            nc.sync.dma_start(
                embd0[:].rearrange("p (s r) -> p s r", s=S_SUP),
                embd_rays[:].unsqueeze(1)
                .broadcast_to([EMB_D, S_SUP, R_CORE]))
            emb_next = (E0, embd0)
            emb_gen = None
            for sti in range(nsup_exec):
                st = sti % NSUP
                sl = slice(st * FSUP, (st + 1) * FSUP)
                E, embd = emb_next
                emb_gen = (emb_stages((sti + 1) % NSUP)
                           if sti + 1 < nsup_exec else None)

                xa = apool.tile([128, 2 * FSUP], F32R, name="xa")
                xb = apool.tile([128, 2 * FSUP], F32R, name="xb")
                h = apool.tile([128, FSUP], F32R, name="h")
                osb = opool.tile([1, FSUP], F32, name="osb")
                rgbsb = opool.tile([3, FSUP], F32, name="rgbsb")

                def xsl(t, chunk, sub):
                    return t[:, chunk * FSUP + sub * F:
                             chunk * FSUP + sub * F + F]

                def rhs_of(src, idx, ksz, cur, sub):
                    if src == "E":
                        return E[0:EMB_X, sub * F:(sub + 1) * F]
                    if src == "embd":
                        return embd[0:EMB_D, sub * F:(sub + 1) * F]
                    return xsl(cur, idx, sub)

                cur = None
                # 8 xyz layers
                for li, kparts in enumerate(_XYZ_LAYERS):
                    nxt = xa if li % 2 == 0 else xb
                    for m in range(2):
                        lhs = []
                        for k, (src, idx, ksz) in enumerate(kparts):
                            if li == 0:
                                lt = w0_t[:, m * 128:(m + 1) * 128]
                            elif src == "E":
                                lt = w4e_t[:, m * 128:(m + 1) * 128]
                            else:
                                lt = wmid_t[li][:, idx * 256 + m * 128:
                                                idx * 256 + m * 128 + 128]
                            lhs.append(lt)
                        for sub in range(NSUB):
                            pt = psum.tile([128, F], F32, name="mmps",
                                           tag="mm")
                            for k, (src, idx, ksz) in enumerate(kparts):
                                nc.tensor.matmul(
                                    pt[:], lhs[k][0:ksz, :],
                                    rhs_of(src, idx, ksz, cur, sub),
                                    start=(k == 0),
                                    stop=(k == len(kparts) - 1))
                            evict(pt[:], xsl(nxt, m, sub),
                                  b_t[:, 2 * li + m:2 * li + m + 1])
                    cur = nxt
                    if emb_gen is not None and 1 <= li <= 5:
                        # emit one stage of the next super-tile's embedding
                        # pipeline so the in-order ACT/DVE queues interleave
                        # it with this tile's evictions instead of taking it
                        # as one burst; the final stage returns the tiles
                        r = next(emb_gen)
                        if r is not None:
                            emb_next = r

                # density head + feat layer (both read cur = x7)
                for sub in range(NSUB):
                    ptd = psum.tile([1, F], F32, name="denps", tag="mm")
                    for k in range(2):
                        nc.tensor.matmul(ptd[:], wden_t[:, k:k + 1],
                                         xsl(cur, k, sub),
                                         start=(k == 0), stop=(k == 1))
                    nc.scalar.activation(osb[0:1, sub * F:(sub + 1) * F],
                                         ptd[:], AF.Relu,
                                         bias=b_t[0:1, 19:20])
                nxt = xa if cur is xb else xb  # feat output
                for m in range(2):
                    for sub in range(NSUB):
                        pt = psum.tile([128, F], F32, name="featps", tag="mm")
                        for k in range(2):
                            nc.tensor.matmul(
                                pt[:],
                                wfeat_t[:, k * 256 + m * 128:
                                        k * 256 + m * 128 + 128],
                                xsl(cur, k, sub),
                                start=(k == 0), stop=(k == 1))
                        evict(pt[:], xsl(nxt, m, sub),
                              b_t[:, 16 + m:17 + m])
                cur = nxt

                # direction layer -> h
                for sub in range(NSUB):
                    pt = psum.tile([128, F], F32, name="dirps", tag="mm")
                    nc.tensor.matmul(pt[:], wd0_t[:, 0:128],
                                     xsl(cur, 0, sub), start=True, stop=False)
                    nc.tensor.matmul(pt[:], wd0_t[:, 128:256],
                                     xsl(cur, 1, sub), start=False, stop=False)
                    nc.tensor.matmul(pt[:], wd0e_t[:],
                                     embd[0:EMB_D, sub * F:(sub + 1) * F],
                                     start=False, stop=True)
                    evict(pt[:], h[:, sub * F:(sub + 1) * F],
                          b_t[:, 18:19])

                # rgb head
                for sub in range(NSUB):
                    ptr = psum.tile([3, F], F32, name="rgbps", tag="mm")
                    nc.tensor.matmul(ptr[:], wrgb_t[:],
                                     h[:, sub * F:(sub + 1) * F],
                                     start=True, stop=True)
                    nc.scalar.activation(rgbsb[:, sub * F:(sub + 1) * F],
                                         ptr[:], AF.Sigmoid,
                                         bias=b_t[0:3, 20:21])

                nc.sync.dma_start(out[0:1, sl], osb[:])
                nc.sync.dma_start(out[1:4, sl], rgbsb[:])

    nc.compile()
    _cache[key] = nc
    return nc


def _prep_inputs(inputs):
    """Host-side shard + transpose prep. Returns list of per-core dicts."""
    f32 = np.float32
    sp = np.ascontiguousarray(inputs["sample_points"], dtype=f32)
    dirs = np.ascontiguousarray(inputs["directions"], dtype=f32)

    dirs_all = dirs.T.copy()                          # [3, 4096]

    def wt(w):  # [out, in] -> [in, out]
        return np.ascontiguousarray(w.T, dtype=f32)

    def wmid_pack(w):  # [256, 256] -> [128, 512] (k-chunk blocks)
        t = wt(w)                                     # [256, 256]
        return np.ascontiguousarray(
            t.reshape(2, 128, 256).transpose(1, 0, 2).reshape(128, 512))

    shared = {}
    shared["w0"] = wt(inputs["Wx0"])                  # [63, 256]
    for i in range(1, 8):
        w = inputs[f"Wx{i}"]
        if i == 4:
            shared["wmid4"] = wmid_pack(w[:, :256])
            shared["w4e"] = wt(w[:, 256:])            # [63, 256]
        else:
            shared[f"wmid{i}"] = wmid_pack(w)
    shared["wfeat"] = wmid_pack(inputs["Wfeat"])
    wden_t = wt(inputs["Wden"])                       # [256, 1]
    shared["wden"] = np.ascontiguousarray(
        wden_t.reshape(2, 128, 1).transpose(1, 0, 2).reshape(128, 2))
    wd0_t = wt(inputs["Wd0"])                         # [283, 128]
    shared["wd0"] = np.ascontiguousarray(
        wd0_t[:256].reshape(2, 128, 128).transpose(1, 0, 2).reshape(128, 256))
    shared["wd0e"] = np.ascontiguousarray(wd0_t[256:])  # [27, 128]
    shared["wrgb"] = wt(inputs["Wrgb"])               # [128, 3]

    bias = np.zeros((128, 21), dtype=f32)
    for li in range(8):
        b = inputs[f"bx{li}"]
        bias[:, 2 * li] = b[:128]
        bias[:, 2 * li + 1] = b[128:]
    bias[:, 16] = inputs["bfeat"][:128]
    bias[:, 17] = inputs["bfeat"][128:]
    bias[:, 18] = inputs["bd0"]
    bias[0, 19] = inputs["bden"][0]
    bias[0:3, 20] = inputs["brgb"]

    consts = np.zeros((128, 4), dtype=f32)
    consts[0:30, 0] = 2.0 ** (np.arange(30) % 10)
    consts[30:60, 0] = 2.0 ** (np.arange(30) % 10)
    consts[30:60, 1] = np.pi / 2
    consts[0:12, 2] = 2.0 ** (np.arange(12) % 4)
    consts[12:24, 2] = 2.0 ** (np.arange(12) % 4)
    consts[12:24, 3] = np.pi / 2

    in_maps = []
    for c in range(N_CORES):
        m = dict(shared)
        # sample-major: [3, S, R] flattened to [3, NPTS]
        blk = sp[c * R_CORE:(c + 1) * R_CORE]         # [R, S, 3]
        m["pts"] = np.ascontiguousarray(
            blk.transpose(2, 1, 0).reshape(3, NPTS))
        m["dirs"] = np.ascontiguousarray(
            dirs_all[:, c * R_CORE:(c + 1) * R_CORE])
        m["biases"] = bias
        m["consts"] = consts
        in_maps.append(m)
    return in_maps


def kernel(**inputs) -> np.ndarray:
    nc = _build()
    in_maps = _prep_inputs(inputs)
    res = run_bass_kernel_spmd(nc, in_maps, core_ids=list(range(N_CORES)))
    outs = []
    for c in range(N_CORES):
        o = res.results[c]["out"]                     # [4, NPTS] sample-major
        outs.append(o.reshape(4, S, R_CORE).transpose(2, 1, 0))
    return np.concatenate(outs, axis=0)



# revision 25
# speedup vs baseline: 1.1913x; 1.1913x over previous
"""NeRF MLP forward pass on 8 Trainium2 NeuronCores (Bass/Tile), fp8 edition.

Strategy: pure data parallel over rays (512 rays x 64 samples = 32768 points
per core, sample-major). All matmuls run in fp8(e4m3) with the DoubleRow perf
mode, which contracts K=256 (two 128-row k-tiles packed along a free dim) at
0.5 PE cycles per output column -- 4x the fp32r rate for the 256-wide hidden
layers. PSUM accumulates in fp32; evictions (relu+bias+fp8-quantize) are
spread across the ACT, DVE and Pool(GPSIMD) engines, which are the bottleneck
in this regime (PE ~11us vs ~17us of eviction work per 2048-point super-tile).

Harmonic embeddings: P = x*(f/2pi) + phase + 512.5 on DVE (per-partition
consts), F = mod(P,1)-0.5 on DVE, then one ACT Sin op (scale=2pi) producing
fp8 directly; a DMA shuffle packs the [120,1024] sin block into the k-tile
layout [32,2,2048] the DoubleRow matmuls need. The per-ray direction
embedding is computed once and broadcast per super-tile by DMA.
"""

import sys

if '/opt/trn_rl_repo' not in sys.path:
    sys.path.insert(0, '/opt/trn_rl_repo')

import numpy as np
import ml_dtypes

import concourse.bacc as bacc
import concourse.mybir as mybir
import concourse.tile as tile
from concourse.bass_utils import run_bass_kernel_spmd

F32 = mybir.dt.float32
FP8 = mybir.dt.float8e4
NP8 = ml_dtypes.float8_e4m3
AF = mybir.ActivationFunctionType
ALU = mybir.AluOpType
DR = mybir.MatmulPerfMode.DoubleRow

N_CORES = 8
N_RAYS, S = 4096, 64
R_CORE = N_RAYS // N_CORES            # 512 rays per core
NPTS = R_CORE * S                     # 32768 points per core
# Sample-major point order: point index = s * R_CORE + r, so a 512-point
# sub-tile is one sample across all rays and the direction embedding
# broadcast is a contiguous block repeat.
F = 512                               # points per matmul (one PSUM bank)
FSUP = 2048                           # points per super-tile
NSUB = FSUP // F                      # 4
NSUP = NPTS // FSUP                   # 16
S_SUP = FSUP // R_CORE                # 4 samples per super-tile
HALF = 1024                           # embedding pipeline column block

PI = float(np.pi)
TWO_PI = 2.0 * PI
INV2PI = float(1.0 / TWO_PI)
MAGIC = float(1.5 * 2 ** 23)          # fp32 round-to-nearest-int trick

_cache = {}


def _rot_seq(n, wa, wd, wp):
    """Weighted largest-remainder interleave of ('A','D','P') engines."""
    targets = {"A": float(wa), "D": float(wd), "P": float(wp)}
    tot = sum(targets.values())
    acc = {k: 0.0 for k in targets}
    seq = []
    for _ in range(n):
        for k in targets:
            acc[k] += targets[k] / tot
        pick = max(acc, key=lambda k: acc[k])
        acc[pick] -= 1.0
        seq.append(pick)
    return seq


def _build(nsup_exec=NSUP):
    key = ("nc", nsup_exec)
    if key in _cache:
        return _cache[key]

    nc = bacc.Bacc("TRN2", target_bir_lowering=False, debug=False,
                   num_devices=N_CORES)

    # pts20 rows 0-59: each coord replicated 10x (for the 60 harmonic rows,
    # duplicated host-side so the P load is one plain DMA); rows 60-62: xyz
    pts20 = nc.dram_tensor("pts20", [63, NPTS], F32, kind="ExternalInput")
    # dirs24 rows 0-23: coords replicated 4x; rows 24-26: xyz
    dirs24 = nc.dram_tensor("dirs24", [27, R_CORE], F32, kind="ExternalInput")
    w0 = nc.dram_tensor("w0", [32, 2, 256], FP8, kind="ExternalInput")
    wmid = {i: nc.dram_tensor(f"wmid{i}", [128, 2, 256], FP8,
                              kind="ExternalInput")
            for i in range(1, 8)}
    w4e = nc.dram_tensor("w4e", [32, 2, 256], FP8, kind="ExternalInput")
    wfeat = nc.dram_tensor("wfeat", [128, 2, 256], FP8, kind="ExternalInput")
    wden = nc.dram_tensor("wden", [128, 2, 32], FP8, kind="ExternalInput")
    wdir = nc.dram_tensor("wdir", [128, 2, 128], FP8, kind="ExternalInput")
    wdire = nc.dram_tensor("wdire", [16, 2, 128], FP8, kind="ExternalInput")
    wrgb = nc.dram_tensor("wrgb", [128, 32], FP8, kind="ExternalInput")
    biases = nc.dram_tensor("biases", [128, 21], F32, kind="ExternalInput")
    consts = nc.dram_tensor("consts", [128, 4], F32, kind="ExternalInput")
    out = nc.dram_tensor("out", [4, NPTS], F32, kind="ExternalOutput")

    with tile.TileContext(nc) as tc:
        with (
            tc.tile_pool(name="wpool", bufs=1) as wpool,
            tc.tile_pool(name="spool", bufs=2) as spool,
            tc.tile_pool(name="epool", bufs=2) as epool,
            tc.tile_pool(name="apool", bufs=1) as apool,
            tc.tile_pool(name="opool", bufs=2) as opool,
            tc.tile_pool(name="psumB", bufs=3, space="PSUM") as psumB,
            tc.tile_pool(name="psumS", bufs=1, space="PSUM") as psumS,
        ):
            # ---- persistent weights / constants ----
            w0_t = wpool.tile([32, 2, 256], FP8)
            nc.sync.dma_start(w0_t[:], w0[:])
            wmid_t = {}
            for i in range(1, 8):
                wt = wpool.tile([128, 2, 256], FP8, name=f"wmid{i}_t")
                nc.sync.dma_start(wt[:], wmid[i][:])
                wmid_t[i] = wt
            w4e_t = wpool.tile([32, 2, 256], FP8)
            nc.sync.dma_start(w4e_t[:], w4e[:])
            wfeat_t = wpool.tile([128, 2, 256], FP8)
            nc.sync.dma_start(wfeat_t[:], wfeat[:])
            wden_t = wpool.tile([128, 2, 32], FP8)
            nc.sync.dma_start(wden_t[:], wden[:])
            wdir_t = wpool.tile([128, 2, 128], FP8)
            nc.sync.dma_start(wdir_t[:], wdir[:])
            wdire_t = wpool.tile([16, 2, 128], FP8)
            nc.sync.dma_start(wdire_t[:], wdire[:])
            wrgb_t = wpool.tile([128, 32], FP8)
            nc.sync.dma_start(wrgb_t[:], wrgb[:])
            b_t = wpool.tile([128, 21], F32)
            nc.sync.dma_start(b_t[:], biases[:])
            c_t = wpool.tile([128, 4], F32)
            nc.sync.dma_start(c_t[:], consts[:])

            # ---- direction embedding per ray (once per core) ----
            # pd rows h*12 + c*4 + k  (h: sin/cos, c: coord, k: freq)
            pd = wpool.tile([24, R_CORE], F32)
            dstage = wpool.tile([3, R_CORE], F32)
            nc.sync.dma_start(dstage[:], dirs24[24:27, :])
            nc.sync.dma_start(pd[0:12, :], dirs24[0:12, :])
            nc.sync.dma_start(pd[12:24, :], dirs24[12:24, :])
            nc.vector.tensor_scalar(pd[:], pd[:], c_t[0:24, 2:3],
                                    c_t[0:24, 3:4], op0=ALU.mult, op1=ALU.add)
            kd = wpool.tile([24, R_CORE], F32)
            nc.vector.tensor_scalar(kd[:], pd[:], MAGIC, MAGIC,
                                    op0=ALU.add, op1=ALU.subtract)
            nc.vector.tensor_tensor(pd[:], pd[:], kd[:], op=ALU.subtract)
            sd = wpool.tile([24, R_CORE], FP8)
            nc.scalar.activation(sd[:], pd[:], AF.Sin, bias=0.0, scale=TWO_PI)
            # pack k-tile layout [16, 2, R]: t0 = rows 0-15, t1 = rows 16-23
            # + xyz rows 24-26 at slots 8-10, zero pad slots 11-15.
            # Engine ops need partition base % 32 == 0, so stage the fp8 xyz
            # cast at base 0 and place rows with DMA.
            embd_rays = wpool.tile([16, 2, R_CORE], FP8)
            nc.gpsimd.memset(embd_rays[:], 0.0)
            dx8 = wpool.tile([3, R_CORE], FP8)
            nc.vector.tensor_scalar(dx8[:], dstage[:], 1.0, None, op0=ALU.mult)
            nc.sync.dma_start(embd_rays[0:16, 0, :], sd[0:16, :])
            nc.sync.dma_start(embd_rays[0:8, 1, :], sd[16:24, :])
            nc.sync.dma_start(embd_rays[8:11, 1, :], dx8[:])

            # ---- super-tile embedding pipeline (generator, interleaved) ----
            def emb_stages(st):
                sl = slice(st * FSUP, (st + 1) * FSUP)
                # P rows 0-59: harmonic rows for points [0,1024); rows 64-123:
                # for points [1024,2048). Junk rows 60-63 flow through
                # harmlessly (never shuffled into E).
                P = spool.tile([128, HALF], F32, name="P")
                if st < 2:
                    nc.gpsimd.memset(P[:], 0.0)
                for h in range(2):
                    slh = slice(st * FSUP + h * HALF,
                                st * FSUP + (h + 1) * HALF)
                    nc.sync.dma_start(P[64 * h:64 * h + 60, :],
                                      pts20[0:60, slh])
                st3 = spool.tile([3, FSUP], F32, name="st3")
                nc.sync.dma_start(st3[:], pts20[60:63, sl])
                embd = epool.tile([16, 2, FSUP], FP8, name="embd")
                for t in range(2):
                    nc.sync.dma_start(
                        embd[:, t, :].rearrange("p (s r) -> p s r", s=S_SUP),
                        embd_rays[:, t, :].unsqueeze(1)
                        .broadcast_to([16, S_SUP, R_CORE]))
                yield None
                nc.gpsimd.tensor_scalar(P[0:124, :], P[0:124, :],
                                        c_t[0:124, 0:1], c_t[0:124, 1:2],
                                        op0=ALU.mult, op1=ALU.add)
                yield None
                Fr = spool.tile([128, HALF], F32, name="Fr")
                nc.gpsimd.tensor_scalar(Fr[0:124, :], P[0:124, :], MAGIC,
                                        MAGIC, op0=ALU.add, op1=ALU.subtract)
                yield None
                nc.gpsimd.tensor_tensor(Fr[0:124, :], P[0:124, :],
                                        Fr[0:124, :], op=ALU.subtract)
                yield None
                Sx = spool.tile([128, HALF], FP8, name="Sx")
                nc.scalar.activation(Sx[0:124, :], Fr[0:124, :], AF.Sin,
                                     bias=0.0, scale=TWO_PI)
                yield None
                # E k-tile layout [32, 2, FSUP]: t0 = harmonic rows 0-31,
                # t1 = rows 32-59 + xyz rows at slots 28-30 + zero pad slot 31
                E = epool.tile([32, 2, FSUP], FP8, name="E")
                nc.sync.dma_start(E[0:32, 0, 0:HALF], Sx[0:32, :])
                nc.sync.dma_start(E[0:28, 1, 0:HALF], Sx[32:60, :])
                nc.sync.dma_start(E[0:32, 0, HALF:FSUP], Sx[64:96, :])
                nc.sync.dma_start(E[0:28, 1, HALF:FSUP], Sx[96:124, :])
                yield None
                # xyz rows: engine ops need base%32==0, so cast at base 0
                # into a staged tile (row 3 kept zero as the k-tile pad) and
                # DMA into E slots 28-31.
                xyz8 = spool.tile([4, FSUP], FP8, name="xyz8")
                if st < 2:
                    nc.gpsimd.memset(xyz8[:], 0.0)
                nc.gpsimd.tensor_scalar(xyz8[0:3, :], st3[:], 1.0, None,
                                        op0=ALU.mult)
                nc.sync.dma_start(E[28:32, 1, :], xyz8[:])
                yield (E, embd)

            # ---- eviction engine rotation (Pool can't read PSUM on TRN2,
            # so evictions split between ACT and DVE; Pool runs the SBUF-only
            # embedding pipeline) ----
            rot = _rot_seq(38, 20, 18, 0)
            ev_i = [0]

            def evict(psum_ap, out_ap, bias_ap):
                eng = rot[ev_i[0] % len(rot)]
                ev_i[0] += 1
                if eng == "A":
                    nc.scalar.activation(out_ap, psum_ap, AF.Relu,
                                         bias=bias_ap)
                else:
                    nc.vector.tensor_scalar(out_ap, psum_ap, bias_ap, 0.0,
                                            op0=ALU.add, op1=ALU.max)

            def dr_rhs(t, sub):
                """[128, 2, F] DoubleRow rhs slice of a [128, 2, FSUP] tile."""
                return t[:, :, sub * F:(sub + 1) * F]

            # ---- main loop ----
            gen0 = emb_stages(0)
            emb_next = None
            for r in gen0:
                if r is not None:
                    emb_next = r

            for sti in range(nsup_exec):
                st = sti % NSUP
                sl = slice(st * FSUP, (st + 1) * FSUP)
                E, embd = emb_next
                emb_gen = (emb_stages((sti + 1) % NSUP)
                           if sti + 1 < nsup_exec else None)

                xa = apool.tile([128, 2, FSUP], FP8, name="xa")
                xb = apool.tile([128, 2, FSUP], FP8, name="xb")
                hT = apool.tile([128, FSUP], FP8, name="hT")
                osb = opool.tile([128, F], F32, name="osb")
                rgbsb = opool.tile([128, F], F32, name="rgbsb")

                cur = None
                for li in range(8):
                    nxt = xa if li % 2 == 0 else xb
                    for m in range(2):
                        for g in range(2):
                            pt = psumB.tile([128, 2 * F], F32, name="mmps",
                                            tag="mm")
                            for s in (2 * g, 2 * g + 1):
                                o = pt[:, (s - 2 * g) * F:(s - 2 * g + 1) * F]
                                if li == 0:
                                    nc.tensor.matmul(
                                        o, w0_t[:, :, m * 128:(m + 1) * 128],
                                        dr_rhs(E, s), start=True, stop=True,
                                        perf_mode=DR)
                                elif li == 4:
                                    nc.tensor.matmul(
                                        o, wmid_t[4][:, :, m * 128:(m + 1) * 128],
                                        dr_rhs(cur, s), start=True, stop=False,
                                        perf_mode=DR)
                                    nc.tensor.matmul(
                                        o, w4e_t[:, :, m * 128:(m + 1) * 128],
                                        dr_rhs(E, s), start=False, stop=True,
                                        perf_mode=DR)
                                else:
                                    nc.tensor.matmul(
                                        o, wmid_t[li][:, :, m * 128:(m + 1) * 128],
                                        dr_rhs(cur, s), start=True, stop=True,
                                        perf_mode=DR)
                            evict(pt[:], nxt[:, m, g * HALF:(g + 1) * HALF],
                                  b_t[:, 2 * li + m:2 * li + m + 1])
                    cur = nxt
                    if emb_gen is not None and 1 <= li <= 7:
                        r = next(emb_gen)
                        if r is not None:
                            emb_next = r

                # density head: one DoubleRow matmul per sub-tile, packed
                # into one PSUM bank at partition offsets 0/32/64/96
                ptd = psumS.tile([128, F], F32, name="denps", tag="den")
                # (DoubleRow + tile_position is rejected by the walrus ISA
                # check, so the den head uses plain fp8 k-chunk matmuls with
                # M=32 replicated weight columns: same 512-column stream cost,
                # and the 4 sub-tiles tile all 128 psum partitions so the
                # bank has no uninitialized gaps for the packed eviction)
                for s in range(NSUB):
                    for t in range(2):
                        nc.tensor.matmul(ptd[32 * s:32 * s + 32, :],
                                         wden_t[:, t, :], cur[:, t,
                                         s * F:(s + 1) * F],
                                         start=(t == 0), stop=(t == 1),
                                         tile_position=(0, 32 * s))
                nc.vector.tensor_scalar(osb[0:97, :], ptd[0:97, :],
                                        b_t[0:97, 19:20], 0.0,
                                        op0=ALU.add, op1=ALU.max)
                for s in range(NSUB):
                    nc.sync.dma_start(out[0:1, st * FSUP + s * F:
                                          st * FSUP + (s + 1) * F],
                                      osb[32 * s:32 * s + 1, :])

                # feat layer
                nxt = xa if cur is xb else xb
                for m in range(2):
                    for g in range(2):
                        pt = psumB.tile([128, 2 * F], F32, name="mmps",
                                        tag="mm")
                        for s in (2 * g, 2 * g + 1):
                            o = pt[:, (s - 2 * g) * F:(s - 2 * g + 1) * F]
                            nc.tensor.matmul(
                                o, wfeat_t[:, :, m * 128:(m + 1) * 128],
                                dr_rhs(cur, s), start=True, stop=True,
                                perf_mode=DR)
                        evict(pt[:], nxt[:, m, g * HALF:(g + 1) * HALF],
                              b_t[:, 16 + m:17 + m])
                cur = nxt

                # direction layer -> h
                for g in range(2):
                    pt = psumB.tile([128, 2 * F], F32, name="mmps", tag="mm")
                    for s in (2 * g, 2 * g + 1):
                        o = pt[:, (s - 2 * g) * F:(s - 2 * g + 1) * F]
                        nc.tensor.matmul(o, wdir_t[:], dr_rhs(cur, s),
                                         start=True, stop=False, perf_mode=DR)
                        nc.tensor.matmul(o, wdire_t[:], dr_rhs(embd, s),
                                         start=False, stop=True, perf_mode=DR)
                    evict(pt[:], hT[:, g * HALF:(g + 1) * HALF],
                          b_t[:, 18:19])

                # rgb head: plain fp8 matmuls packed into one bank
                ptr = psumS.tile([128, F], F32, name="rgbps", tag="rgb")
                for s in range(NSUB):
                    nc.tensor.matmul(ptr[32 * s:32 * s + 32, :], wrgb_t[:],
                                     hT[:, s * F:(s + 1) * F],
                                     start=True, stop=True,
                                     tile_position=(0, 32 * s))
                nc.scalar.activation(rgbsb[0:99, :], ptr[0:99, :], AF.Sigmoid,
                                     bias=b_t[0:99, 20:21])
                for s in range(NSUB):
                    nc.sync.dma_start(out[1:4, st * FSUP + s * F:
                                          st * FSUP + (s + 1) * F],
                                      rgbsb[32 * s:32 * s + 3, :])

    nc.compile()
    _cache[key] = nc
    return nc


def _prep_inputs(inputs):
    """Host-side shard + transpose + fp8 weight prep."""
    f32 = np.float32
    sp = np.ascontiguousarray(inputs["sample_points"], dtype=f32)
    dirs_all = np.ascontiguousarray(inputs["directions"], dtype=f32).T  # [3,N]

    def q8(w):
        return np.ascontiguousarray(np.asarray(w, dtype=f32).astype(NP8))

    def wt(w):  # [out, in] -> [in, out]
        return np.ascontiguousarray(np.asarray(w, dtype=f32).T)

    def pack_mid(w):  # [256, K256] -> [128, 2, 256] k-tile layout
        t = wt(w)                                       # [256, 256]
        return q8(t.reshape(2, 128, t.shape[1]).transpose(1, 0, 2))

    def pack_emb(wE):  # [256out, 63in] -> [32, 2, 256]: see E layout
        t = wt(wE)                                      # [63, 256]
        arr = np.zeros((32, 2, t.shape[1]), dtype=f32)
        arr[:, 0, :] = t[0:32]
        arr[0:28, 1, :] = t[32:60]
        arr[28:31, 1, :] = t[60:63]                     # xyz rows
        return q8(arr)

    shared = {}
    shared["w0"] = pack_emb(inputs["Wx0"])
    for i in range(1, 8):
        w = np.asarray(inputs[f"Wx{i}"], dtype=f32)
        if i == 4:
            shared["wmid4"] = pack_mid(w[:, :256])
            shared["w4e"] = pack_emb(w[:, 256:])
        else:
            shared[f"wmid{i}"] = pack_mid(w)
    shared["wfeat"] = pack_mid(inputs["Wfeat"])
    shared["wden"] = np.ascontiguousarray(np.broadcast_to(
        pack_mid(inputs["Wden"]).reshape(128, 2, 1), (128, 2, 32)))
    wd0 = np.asarray(inputs["Wd0"], dtype=f32)          # [128, 283]
    shared["wdir"] = pack_mid(wd0[:, :256])
    wde = wt(wd0[:, 256:283])                           # [27, 128]
    arr = np.zeros((16, 2, 128), dtype=f32)
    arr[:, 0, :] = wde[0:16]
    arr[0:8, 1, :] = wde[16:24]
    arr[8:11, 1, :] = wde[24:27]                        # xyz rows
    shared["wdire"] = q8(arr)
    wrgb3 = wt(inputs["Wrgb"])                          # [128, 3]
    shared["wrgb"] = q8(np.concatenate(
        [np.tile(wrgb3, (1, 10)), wrgb3[:, 0:2]], axis=1))  # [128, 32]

    bias = np.zeros((128, 21), dtype=f32)
    for li in range(8):
        b = np.asarray(inputs[f"bx{li}"], dtype=f32)
        bias[:, 2 * li] = b[:128]
        bias[:, 2 * li + 1] = b[128:]
    bias[:, 16] = np.asarray(inputs["bfeat"], dtype=f32)[:128]
    bias[:, 17] = np.asarray(inputs["bfeat"], dtype=f32)[128:]
    bias[:, 18] = np.asarray(inputs["bd0"], dtype=f32)
    for s in range(4):
        bias[32 * s, 19] = float(np.asarray(inputs["bden"], dtype=f32)[0])
        bias[32 * s:32 * s + 3, 20] = np.asarray(inputs["brgb"], dtype=f32)
    shared["biases"] = bias

    # consts: col0/1 xyz pipeline (rows 0-59 and 64-123), col2/3 dirs
    # P = x*(f/2pi) + phase'; F = P - round(P) (MAGIC trick) so
    # sin(2pi*F) = sin(x*f + 2pi*phase'). phase' = 1/4 turn for cos rows.
    # (No large additive offset here: with round-to-nearest any non-integer
    # offset would phase-shift the result.)
    consts = np.zeros((128, 4), dtype=f32)
    fr = (2.0 ** (np.arange(60) % 10)) / (2.0 * np.pi)
    ph = 0.25 * (np.arange(60) >= 30)
    consts[0:60, 0] = fr
    consts[64:124, 0] = fr
    consts[0:60, 1] = ph
    consts[64:124, 1] = ph
    consts[0:24, 2] = (2.0 ** (np.arange(24) % 4)) / (2.0 * np.pi)
    consts[0:24, 3] = 0.25 * (np.arange(24) >= 12)
    shared["consts"] = consts

    in_maps = []
    for c in range(N_CORES):
        m = dict(shared)
        blk = sp[c * R_CORE:(c + 1) * R_CORE]           # [R, S, 3]
        pts = blk.transpose(2, 1, 0).reshape(3, NPTS)   # sample-major
        p20 = np.empty((63, NPTS), dtype=f32)
        p20[0:30] = np.repeat(pts, 10, axis=0)          # sin rows
        p20[30:60] = p20[0:30]                          # cos rows
        p20[60:63] = pts
        m["pts20"] = p20
        d = dirs_all[:, c * R_CORE:(c + 1) * R_CORE]    # [3, R]
        d24 = np.empty((27, R_CORE), dtype=f32)
        d24[0:12] = np.repeat(d, 4, axis=0)
        d24[12:24] = d24[0:12]
        d24[24:27] = d
        m["dirs24"] = d24
        in_maps.append(m)
    return in_maps


def kernel(**inputs) -> np.ndarray:
    nc = _build()
    in_maps = _prep_inputs(inputs)
    res = run_bass_kernel_spmd(nc, in_maps, core_ids=list(range(N_CORES)))
    outs = []
    for c in range(N_CORES):
        o = res.results[c]["out"]                       # [4, NPTS] sample-major
        outs.append(o.reshape(4, S, R_CORE).transpose(2, 1, 0))
    return np.concatenate(outs, axis=0)
